# revision 47
# baseline (speedup 1.0000x reference)
"""Self-contained Trainium2 Bass kernel: pre-LN multi-head attention block.

Computes, for x [B=8, S=1024, D=1024] (fp32) and packed attention weights:
    out = x + out_proj(MHA(LayerNorm(x)))
matching torch nn.MultiheadAttention's explicit (non-flash) path with 16 heads.

Sharding: data-parallel over batch - core i handles batch element i; no
collectives, outputs are concatenated on the host.

Per-core strategy (fp8 DoubleRow matmuls at 2x PE throughput):
  - LN runs on transposed activations (d on partitions); stats are matmuls
    against an all-ones stationary so the sums land partition-replicated in
    PSUM; the normalize chain runs in bf16 on DVE and the gamma/beta apply
    runs on the Pool engine (tensor_scalar), writing xn directly in fp8.
  - QKV / V / PV / out-proj all run as fp8e4 DoubleRow matmuls with full
    128-wide stationaries: weights are pre-scaled by 32 on the host (power
    of two; folded back out via the softmax exp scale and the final output
    scale); each instruction contracts 2x128 d-coords at 0.5 cycles per
    output column.
  - scores^T[t,s] = K^T.T @ Q^T per head stay bf16 (K=64 contraction gains
    nothing from DoubleRow); exp runs on the scalar engine over [128, 1024]
    PSUM tiles (amortizing the fixed ACT access latency) with scale 1/8192
    and a -3 offset (cancels in softmax; keeps fp8 exp in range).
  - the softmax denominator comes from a DoubleRow matmul against an fp8
    all-ones stationary - its [64, N] output is the denominator replicated
    across 64 partitions, so the per-head normalize needs no broadcast.
  - PE emission: Q/K/V units are split by s-half so the first halves (plus
    warmup matmuls) keep the PE busy while LayerNorm finishes the second
    x chunk; per head pair the stream is qk(p+1) | scores(p) | pv+den(p-1)
    so the scalar engine's exp stream overlaps PE work throughout.
  - PSUM->SBUF copies alternate between DVE and the otherwise-idle Pool
    engine; residual + out_proj bias are pre-added on the host (bf16) and
    merged with one fused scalar_tensor_tensor: (psum * 2^-10) + resid.
"""

import numpy as np
import ml_dtypes

P = 128
D = 1024
H = 16
DH = 64
B = 8
S = 1024
LN_EPS = 1e-5
N_CORES = 8

_ND = D // P   # d tiles (8)
NS = S // P    # s tiles (8)
NCK = 512      # LN chunk / matmul moving width
WS = 32.0      # fp8 weight pre-scale (power of two)
EXP_SCALE = 0.125 / (WS * WS)   # 1/8192: folds 1/sqrt(dh) and the q/k scales
EXP_BIAS = -3.0                 # cancels in softmax; keeps fp8 exp in range
OUT_SCALE = 1.0 / (WS * WS)     # folds the v/out-proj weight scales back out

POOL_TT = ()

LAST_RESULTS = None
_NC_CACHE = {}


def _emit(tc, aps):
    from concourse import mybir
    from concourse.masks import make_identity

    nc = tc.nc
    f32 = mybir.dt.float32
    bf16 = mybir.dt.bfloat16
    fp8 = mybir.dt.float8e4
    FT = mybir.ActivationFunctionType
    OP = mybir.AluOpType
    DR = mybir.MatmulPerfMode.DoubleRow

    x8d, resid, wqkt, wvt, woutt, negw1qk, negw1v, bq32, binv, out = (
        aps["x8"], aps["resid"], aps["wqkt"], aps["wvt"], aps["woutt"],
        aps["negw1qk"], aps["negw1v"], aps["bq32"], aps["binv"], aps["out"],
    )

    with tc.tile_pool(name="consts", bufs=1) as consts, \
         tc.tile_pool(name="acts", bufs=1) as acts, \
         tc.tile_pool(name="wpool", bufs=1) as wpool:

        # ---------- constants ----------
        ident = consts.tile([P, P], bf16)
        make_identity(nc, ident[:])
        ones_mat = consts.tile([P, P], bf16)
        nc.vector.memset(ones_mat, 1.0)
        ones1f8 = consts.tile([P, 1], fp8)
        nc.vector.memset(ones1f8, 1.0)
        ones1 = consts.tile([P, 1], bf16)
        nc.vector.memset(ones1, 1.0)
        # row-selector stationaries: sel[p, r, f] = (p == r)
        sel = consts.tile([16, 16, P], bf16)
        nc.gpsimd.memset(sel, 0.0)
        nc.gpsimd.affine_select(
            out=sel, in_=sel, compare_op=mybir.AluOpType.not_equal,
            fill=1.0, base=0, pattern=[[-1, 16], [0, P]], channel_multiplier=1)
        eps_sb = consts.tile([P, 1], f32)
        nc.vector.memset(eps_sb, LN_EPS)
        ebase = consts.tile([P, S], f32)
        nc.vector.memset(ebase, float(np.exp(EXP_SCALE)))
        binv_bc = consts.tile([P, D], f32)
        nw1qk_sb = consts.tile([1, 2 * D], bf16)
        nw1v_sb = consts.tile([1, D], bf16)
        bq32_sb = consts.tile([P, H], fp8)

        # ---------- persistent activations ----------
        x8_sb = acts.tile([P, _ND, S], fp8)      # x^T fp8 (QKV + LN stats)
        qkT = acts.tile([P, 2 * _ND, S], fp8)    # q tiles 0..7, k tiles 8..15
        v8w = acts.tile([P, NS // 2, 2, H, P], fp8)  # [v | ones] stationary
        ctxT8 = acts.tile([P, _ND, S], fp8)      # normalized ctx^T (x32)
        resid_sb = acts.tile([P, NS, D], bf16)   # x + out_proj_b, natural
        stage_sb = acts.tile([P, NS, D], bf16)
        b_bc = acts.tile([P, S], bf16)           # rstd, partition-replicated
        mu_bc = acts.tile([P, S], bf16)          # mu, partition-replicated
        bK_sb = acts.tile([P, H, NS], f32)       # exp bias: 2^-13*(bq.K) - 3
        bKr_sb = acts.tile([P, H, NS], f32)      # raw-scale version (pool exp)
        bS_sb = acts.tile([P, NS], f32)          # rstd per t (V drain scalar)

        # ---------- weights (fp8, pre-scaled by WS on host) ----------
        wqk_sb = wpool.tile([P, _ND, 2 * D], fp8)
        wv_sb = wpool.tile([P, _ND, D], fp8)
        wout_sb = wpool.tile([P, _ND, D], fp8)

        # ones planes of the wide PV stationary (denominator columns)
        nc.gpsimd.memset(v8w[:, :, :, :, DH:P], 1.0)

        # ================= Phase 1: LayerNorm stats =================
        with tc.tile_pool(name="lnps", bufs=1, space="PSUM") as lnps, \
             tc.tile_pool(name="lntmp", bufs=1) as lntmp:
            stats_ps = lnps.tile([P, 2 * NS], f32, tag="stats")
            tps = lnps.tile([16, P], bf16, tag="tps")
            rep_ps = lnps.tile([P, 2 * S], f32, tag="rep")
            warm_ps = lnps.tile([P, P], f32, tag="warm")
            for _ in range(24):
                nc.tensor.matmul(warm_ps, lhsT=ones_mat, rhs=ones_mat,
                                 start=True, stop=True)
            NCK = 512
            x8_r = x8d.rearrange("(a p) s -> p a s", p=P)
            for c in range(2):
                sl = slice(c * NCK, (c + 1) * NCK)
                nc.sync.dma_start(out=x8_sb[:, :, sl], in_=x8_r[:, :, sl])
            wqkt_r2 = wqkt.rearrange("(a p) e -> p a e", p=P)
            nc.sync.dma_start(out=wqk_sb[:, :, 0:2 * P],
                              in_=wqkt_r2[:, :, 0:2 * P])
            nc.sync.dma_start(out=wqk_sb[:, :, 8 * P:10 * P],
                              in_=wqkt_r2[:, :, 8 * P:10 * P])
            nc.sync.dma_start(out=wqk_sb[:, :, 2 * P:8 * P],
                              in_=wqkt_r2[:, :, 2 * P:8 * P])
            nc.sync.dma_start(out=wqk_sb[:, :, 10 * P:16 * P],
                              in_=wqkt_r2[:, :, 10 * P:16 * P])
            nc.scalar.dma_start(out=wv_sb,
                                in_=wvt.rearrange("(a p) e -> p a e", p=P))
            nc.scalar.dma_start(out=wout_sb,
                                in_=woutt.rearrange("(a p) e -> p a e", p=P))
            nc.gpsimd.dma_start(out=resid_sb,
                                in_=resid.rearrange("(st p) e -> p st e", p=P))
            nc.gpsimd.dma_start(out=binv_bc,
                                in_=binv[None, :].to_broadcast((P, D)))
            nc.gpsimd.dma_start(out=nw1qk_sb, in_=negw1qk)
            nc.gpsimd.dma_start(out=nw1v_sb, in_=negw1v)
            nc.gpsimd.dma_start(out=bq32_sb, in_=bq32)

            # squares (DVE) + 1-col stats matmuls; accumulation groups must
            # be strictly sequential within the psum bank
            for c in range(2):
                sqs = []
                with nc.allow_low_precision(reason="x^2 for LN stats"):
                    for j in range(_ND):
                        sl = slice(c * NCK, (c + 1) * NCK)
                        sq = lntmp.tile([P, NCK], bf16, tag="sq", bufs=16,
                                        name=f"sq{c}_{j}")
                        nc.vector.tensor_tensor(out=sq, in0=x8_sb[:, j, sl],
                                                in1=x8_sb[:, j, sl],
                                                op=OP.mult)
                        sqs.append(sq)
                for st in range(c * 4, c * 4 + 4):
                    o = st * P - c * NCK
                    for j in range(_ND):
                        nc.tensor.matmul(stats_ps[:, st:st + 1],
                                         lhsT=x8_sb[:, j, st * P:(st + 1) * P],
                                         rhs=ones1f8,
                                         start=(j == 0), stop=(j == _ND - 1))
                    for j in range(_ND):
                        nc.tensor.matmul(stats_ps[:, NS + st:NS + st + 1],
                                         lhsT=sqs[j][:, o:o + P], rhs=ones1,
                                         start=(j == 0), stop=(j == _ND - 1))

            # chain on [128, 8] tiles: partition = s%128, col = s-tile
            ch = lntmp.tile([P, 6 * NS], f32, tag="ch")
            mu = ch[:, 0:NS]
            var = ch[:, NS:2 * NS]
            std = ch[:, 2 * NS:3 * NS]
            b8f = ch[:, 3 * NS:4 * NS]
            nc.vector.tensor_scalar_mul(mu, stats_ps[:, 0:NS], 1.0 / D)
            with nc.allow_low_precision(reason="LN chain"):
                musq = ch[:, 4 * NS:5 * NS]
                nc.vector.tensor_tensor(out=musq, in0=mu, in1=mu, op=OP.mult)
                nc.vector.scalar_tensor_tensor(
                    out=var, in0=stats_ps[:, NS:2 * NS], scalar=1.0 / D,
                    in1=musq, op0=OP.mult, op1=OP.subtract)
            nc.scalar.activation(out=std, in_=var, func=FT.Sqrt, bias=eps_sb)
            nc.vector.reciprocal(out=b8f, in_=std)
            nc.vector.tensor_copy(out=bS_sb, in_=b8f)
            bmu = lntmp.tile([P, 16], bf16, tag="bmu")
            with nc.allow_low_precision(reason="LN stats to bf16"):
                nc.vector.tensor_copy(out=bmu[:, 0:NS], in_=b8f)
                nc.vector.tensor_copy(out=bmu[:, NS:16], in_=mu)
            nc.tensor.transpose(out=tps, in_=bmu, identity=ident)
            tsb = lntmp.tile([16, P], bf16, tag="tsb")
            nc.vector.tensor_copy(out=tsb, in_=tps)
            # replicate b and mu across partitions via selector matmuls
            for st in range(NS):
                nc.tensor.matmul(rep_ps[:, st * P:(st + 1) * P],
                                 lhsT=sel[:, st, :], rhs=tsb,
                                 start=True, stop=True)
                nc.tensor.matmul(rep_ps[:, S + st * P:S + (st + 1) * P],
                                 lhsT=sel[:, NS + st, :], rhs=tsb,
                                 start=True, stop=True)
            with nc.allow_low_precision(reason="LN bcast to bf16"):
                nc.vector.tensor_copy(out=b_bc, in_=rep_ps[:, 0:S])
                nc.vector.tensor_copy(out=mu_bc, in_=rep_ps[:, S:2 * S])

        # ============ Phases 2-4: projections + attention + out-proj ========
        with tc.tile_pool(name="expool", bufs=1) as expool, \
             tc.tile_pool(name="sidep", bufs=1) as sidep, \
             tc.tile_pool(name="mps", bufs=1, space="PSUM") as mps:

            def dr_matmul(ps_out, lhsT, rhs, start, stop):
                nc.tensor.matmul(ps_out, lhsT=lhsT, rhs=rhs, start=start,
                                 stop=stop, perf_mode=DR)

            def emit_qk_half(et, half):
                # one q|k e-tile s-half: 4 DR passes + mu*w1 correction,
                # then one drain (x rstd via b_bc) straight to fp8
                e0 = et * P
                ps = mps.tile([P, NCK], f32, tag="mm", bufs=2,
                              name=f"qk{et}_{half}")
                sl = slice(half * NCK, (half + 1) * NCK)
                for c2 in range(2):
                    s2 = slice(half * NCK + c2 * 256,
                               half * NCK + (c2 + 1) * 256)
                    for jp in range(_ND // 2):
                        dr_matmul(ps[:, c2 * 256:(c2 + 1) * 256],
                                  wqk_sb[:, 2 * jp:2 * jp + 2, e0:e0 + P],
                                  x8_sb[:, 2 * jp:2 * jp + 2, s2],
                                  start=(jp == 0), stop=False)
                    nc.tensor.matmul(ps[:, c2 * 256:(c2 + 1) * 256],
                                     lhsT=nw1qk_sb[0:1, e0:e0 + P],
                                     rhs=mu_bc[0:1, s2],
                                     start=False, stop=True)
                with nc.allow_low_precision(reason="qk to fp8"):
                    nc.vector.tensor_tensor(out=qkT[:, et, sl], in0=ps,
                                            in1=b_bc[:, sl], op=OP.mult)

            def emit_bk(et, tts):
                # exp-bias fold: bK[t, h] = 2^-13*(bq . k)[t] + EXP_BIAS
                bps_full = mps.tile([P, NCK], f32, tag="mm", bufs=2,
                                    name=f"bk{et}_{tts[0]}")
                bps = bps_full[:, 0:2 * NS]
                for idx in range(2):
                    h = 2 * (et - 8) + idx
                    base = idx * DH
                    for tt in tts:
                        nc.tensor.matmul(
                            bps[:, idx * NS + tt:idx * NS + tt + 1],
                            lhsT=qkT[base:base + DH, et, tt * P:(tt + 1) * P],
                            rhs=bq32_sb[base:base + DH, h:h + 1],
                            start=True, stop=True, tile_position=(base, 0))
                lo, hi = tts[0], tts[-1] + 1
                for idx in range(2):
                    h = 2 * (et - 8) + idx
                    sl = slice(idx * NS + lo, idx * NS + hi)
                    nc.vector.tensor_scalar(out=bK_sb[:, h, lo:hi],
                                            in0=bps[:, sl],
                                            scalar1=EXP_SCALE,
                                            scalar2=EXP_BIAS,
                                            op0=OP.mult, op1=OP.add)
                    nc.vector.tensor_scalar(out=bKr_sb[:, h, lo:hi],
                                            in0=bps[:, sl], scalar1=1.0,
                                            scalar2=EXP_BIAS / EXP_SCALE,
                                            op0=OP.mult, op1=OP.add)

            def emit_v_unit(st, eh):
                # V natural [t-tile, (h,dh)] per e-half: 4 DR + mu*w1v fix,
                # drain scales by rstd[t] (per-partition) and adds the bias
                t0 = st * P
                ps = mps.tile([P, NCK], f32, tag="mm", bufs=2,
                              name=f"v{st}_{eh}")
                sl = slice(eh * NCK, (eh + 1) * NCK)
                for c2 in range(2):
                    s2 = slice(eh * NCK + c2 * 256, eh * NCK + (c2 + 1) * 256)
                    for jp in range(_ND // 2):
                        dr_matmul(ps[:, c2 * 256:(c2 + 1) * 256],
                                  x8_sb[:, 2 * jp:2 * jp + 2, t0:t0 + P],
                                  wv_sb[:, 2 * jp:2 * jp + 2, s2],
                                  start=(jp == 0), stop=False)
                    nc.tensor.matmul(ps[:, c2 * 256:(c2 + 1) * 256],
                                     lhsT=mu_bc[0:1, t0:t0 + P],
                                     rhs=nw1v_sb[0:1, s2],
                                     start=False, stop=True)
                with nc.allow_low_precision(reason="v to fp8"):
                    nc.vector.scalar_tensor_tensor(
                        out=v8w[:, st // 2, st % 2, 8 * eh:8 * (eh + 1), 0:DH],
                        in0=ps.rearrange("p (h d) -> p h d", d=DH),
                        scalar=bS_sb[:, st:st + 1],
                        in1=binv_bc[:, sl].rearrange("p (h d) -> p h d", d=DH),
                        op0=OP.mult, op1=OP.add)

            def alloc_ex(hp):
                return expool.tile([P, 2, NS, S], fp8, tag="ex", bufs=2,
                                   name=f"ex{hp}")

            def emit_scores_piece(hp, ex_t, tts, shs):
                for tt in tts:
                    for idx in range(2):
                        h = 2 * hp + idx
                        base = idx * DH
                        ps = mps.tile([P, S], f32, tag="sc", bufs=2,
                                      name=f"sc{hp}_{tt}_{idx}_{shs[0]}")
                        for sh in shs:
                            sl = slice(sh * NCK, (sh + 1) * NCK)
                            nc.tensor.matmul(
                                ps[:, sl],
                                lhsT=qkT[base:base + DH, 8 + hp, tt * P:(tt + 1) * P],
                                rhs=qkT[base:base + DH, hp, sl],
                                start=True, stop=True, tile_position=(base, 0))
                        lo = shs[0] * NCK
                        hi = (shs[-1] + 1) * NCK
                        with nc.allow_low_precision(reason="exp to fp8"):
                            if tt in POOL_TT and shs == (0, 1):
                                sst = sidep.tile([P, S], bf16, tag="sst",
                                                 bufs=2,
                                                 name=f"sst{hp}_{tt}_{idx}")
                                nc.vector.tensor_scalar_add(
                                    sst, ps, bKr_sb[:, h, tt:tt + 1])
                                nc.gpsimd.tensor_tensor(
                                    out=ex_t[:, idx, tt, :], in0=ebase,
                                    in1=sst, op=OP.pow)
                            else:
                                nc.scalar.activation(
                                    out=ex_t[:, idx, tt, lo:hi],
                                    in_=ps[:, lo:hi], func=FT.Exp,
                                    scale=EXP_SCALE,
                                    bias=bK_sb[:, h, tt:tt + 1])

            def emit_scores(hp):
                ex_t = alloc_ex(hp)
                emit_scores_piece(hp, ex_t, range(NS), (0, 1))
                return ex_t

            def emit_pvden(hp, ex_t):
                # PV with wide [v | ones] stationary: den accumulates in psum
                # partitions 64-127 of the same matmuls
                for sh in range(2):
                    for idx in range(2):
                        h = 2 * hp + idx
                        ctxps = mps.tile([P, NCK], f32, tag="ctx", bufs=2,
                                         name=f"ctx{hp}_{sh}_{idx}")
                        for c2 in range(2):
                            for ttp in range(NS // 2):
                                sl = slice(sh * NCK + c2 * 256,
                                           sh * NCK + (c2 + 1) * 256)
                                co = slice(c2 * 256, (c2 + 1) * 256)
                                dr_matmul(ctxps[:, co],
                                          v8w[:, ttp, :, h, :],
                                          ex_t[:, idx, 2 * ttp:2 * ttp + 2, sl],
                                          start=(ttp == 0),
                                          stop=(ttp == NS // 2 - 1))
                        sl = slice(sh * NCK, (sh + 1) * NCK)
                        rden = sidep.tile([DH, NCK], bf16, tag="rd", bufs=4,
                                          name=f"rd{hp}_{sh}_{idx}")
                        with nc.allow_low_precision(reason="denom in bf16"):
                            nc.vector.reciprocal(out=rden, in_=ctxps[DH:P, :])
                            nc.vector.tensor_tensor(
                                out=ctxT8[idx * DH:(idx + 1) * DH, hp, sl],
                                in0=ctxps[0:DH, :], in1=rden, op=OP.mult)

            def emit_outproj_a(sts):
                # heads 0-7 partial -> stage (residual folded in)
                for st in sts:
                    s0 = st * P
                    for eh in range(2):
                        ps = mps.tile([P, NCK], f32, tag="mm", bufs=2,
                                      name=f"opa{st}_{eh}")
                        sl = slice(eh * NCK, (eh + 1) * NCK)
                        for c2 in range(2):
                            s2 = slice(eh * NCK + c2 * 256,
                                       eh * NCK + (c2 + 1) * 256)
                            for hpp in range(2):
                                dr_matmul(ps[:, c2 * 256:(c2 + 1) * 256],
                                          ctxT8[:, 2 * hpp:2 * hpp + 2, s0:s0 + P],
                                          wout_sb[:, 2 * hpp:2 * hpp + 2, s2],
                                          start=(hpp == 0), stop=(hpp == 1))
                        with nc.allow_low_precision(reason="stage in bf16"):
                            nc.vector.scalar_tensor_tensor(
                                out=stage_sb[:, st, sl], in0=ps,
                                scalar=OUT_SCALE, in1=resid_sb[:, st, sl],
                                op0=OP.mult, op1=OP.add)

            def emit_outproj_b():
                for st in range(NS):
                    ps = mps.tile([P, S], f32, tag="sc", bufs=2,
                                  name=f"opb{st}")
                    s0 = st * P
                    for eh in range(2):
                        for c2 in range(2):
                            s2 = slice(eh * NCK + c2 * 256,
                                       eh * NCK + (c2 + 1) * 256)
                            for hpp in range(2, _ND // 2):
                                dr_matmul(ps[:, eh * NCK + c2 * 256:
                                              eh * NCK + (c2 + 1) * 256],
                                          ctxT8[:, 2 * hpp:2 * hpp + 2, s0:s0 + P],
                                          wout_sb[:, 2 * hpp:2 * hpp + 2, s2],
                                          start=(hpp == 2),
                                          stop=(hpp == _ND // 2 - 1))
                    ob = sidep.tile([P, S], bf16, tag="ob", bufs=3,
                                    name=f"ob{st}")
                    with nc.allow_low_precision(reason="out in bf16"):
                        if st < NS - 2:
                            nc.vector.scalar_tensor_tensor(
                                out=ob, in0=ps, scalar=OUT_SCALE,
                                in1=stage_sb[:, st, :], op0=OP.mult,
                                op1=OP.add)
                            nc.sync.dma_start(out=out[st * P:(st + 1) * P, :],
                                              in_=ob)
                        else:
                            for hf in range(2):
                                sl = slice(hf * NCK, (hf + 1) * NCK)
                                nc.vector.scalar_tensor_tensor(
                                    out=ob[:, sl], in0=ps[:, sl],
                                    scalar=OUT_SCALE,
                                    in1=stage_sb[:, st, sl], op0=OP.mult,
                                    op1=OP.add)
                                nc.sync.dma_start(
                                    out=out[st * P:(st + 1) * P, sl],
                                    in_=ob[:, sl])

            # ---- interleaved emission (baseline pacing structure) ----
            emit_qk_half(0, 0)
            emit_qk_half(8, 0)
            emit_bk(8, (0, 1, 2, 3))
            ex_prev = alloc_ex(0)
            emit_scores_piece(0, ex_prev, range(NS // 2), (0,))
            emit_qk_half(0, 1)
            emit_qk_half(8, 1)
            emit_bk(8, (4, 5, 6, 7))
            emit_scores_piece(0, ex_prev, range(NS // 2), (1,))
            emit_scores_piece(0, ex_prev, range(NS // 2, NS), (0, 1))
            for half in range(2):
                emit_qk_half(1, half)
                emit_qk_half(9, half)
            emit_bk(9, (0, 1, 2, 3, 4, 5, 6, 7))
            ex_cur = emit_scores(1)
            for st in range(NS):
                for eh in range(2):
                    emit_v_unit(st, eh)
            emit_pvden(0, ex_prev)
            ex_prev = ex_cur
            for p in range(2, H // 2):
                for half in range(2):
                    emit_qk_half(p, half)
                    emit_qk_half(8 + p, half)
                emit_bk(8 + p, (0, 1, 2, 3, 4, 5, 6, 7))
                ex_cur = emit_scores(p)
                emit_pvden(p - 1, ex_prev)
                ex_prev = ex_cur
                if p >= 4:
                    emit_outproj_a(range(2 * (p - 4), 2 * (p - 3)))
            emit_pvden(H // 2 - 1, ex_prev)
            emit_outproj_b()


def build_nc():
    import concourse.bacc as bacc
    import concourse.tile as tile
    from concourse import mybir

    f32 = mybir.dt.float32
    bf16 = mybir.dt.bfloat16
    fp8 = mybir.dt.float8e4

    nc = bacc.Bacc("TRN2", target_bir_lowering=False, debug=False)
    aps = {
        "x8": nc.dram_tensor("x8", [D, S], fp8, kind="ExternalInput").ap(),
        "resid": nc.dram_tensor("resid", [S, D], bf16, kind="ExternalInput").ap(),
        "wqkt": nc.dram_tensor("wqkt", [D, 2 * D], fp8, kind="ExternalInput").ap(),
        "wvt": nc.dram_tensor("wvt", [D, D], fp8, kind="ExternalInput").ap(),
        "woutt": nc.dram_tensor("woutt", [D, D], fp8, kind="ExternalInput").ap(),
        "negw1qk": nc.dram_tensor("negw1qk", [1, 2 * D], bf16, kind="ExternalInput").ap(),
        "negw1v": nc.dram_tensor("negw1v", [1, D], bf16, kind="ExternalInput").ap(),
        "bq32": nc.dram_tensor("bq32", [P, H], fp8, kind="ExternalInput").ap(),
        "binv": nc.dram_tensor("binv", [D], f32, kind="ExternalInput").ap(),
        "out": nc.dram_tensor("out", [S, D], bf16, kind="ExternalOutput").ap(),
    }
    with tile.TileContext(nc) as tc:
        _emit(tc, aps)
    nc.compile()
    return nc


def prep_inputs(x, ln_gamma, ln_beta, in_proj_w, in_proj_b, out_proj_w, out_proj_b,
                n_cores=N_CORES):
    bf = ml_dtypes.bfloat16
    f8 = ml_dtypes.float8_e4m3
    win = np.asarray(in_proj_w, np.float32)
    g = np.asarray(ln_gamma, np.float32)
    bt = np.asarray(ln_beta, np.float32)
    bin_ = np.asarray(in_proj_b, np.float32)
    wing = win * g[None, :]          # gamma folded into in-proj columns
    binf = bin_ + win @ bt           # beta folded into the in-proj biases
    wqkt8 = np.ascontiguousarray((wing[:2 * D] * WS).T).astype(f8)
    wvt8 = np.ascontiguousarray((wing[2 * D:] * WS).T).astype(f8)
    negw1qk = -wqkt8.astype(np.float32).sum(axis=0, keepdims=True)
    negw1v = -wvt8.astype(np.float32).sum(axis=0, keepdims=True)
    bq32 = np.zeros((P, H), np.float32)
    for h in range(H):
        bq32[(h % 2) * DH:(h % 2) * DH + DH, h] = WS * binf[h * DH:(h + 1) * DH]
    shared = {
        "wqkt": wqkt8,
        "wvt": wvt8,
        "woutt": np.ascontiguousarray(np.asarray(out_proj_w, np.float32).T * WS).astype(f8),
        "negw1qk": negw1qk.astype(bf),
        "negw1v": negw1v.astype(bf),
        "bq32": bq32.astype(f8),
        "binv": np.ascontiguousarray(binf[2 * D:] * WS, np.float32),
    }
    bout = np.asarray(out_proj_b, np.float32)
    in_maps = []
    for i in range(n_cores):
        xi = np.asarray(x[i], np.float32)
        m = dict(shared)
        m["x8"] = np.ascontiguousarray(xi.T).astype(f8)
        m["resid"] = np.ascontiguousarray(xi + bout).astype(bf)
        in_maps.append(m)
    return in_maps


def kernel(x, ln_gamma, ln_beta, in_proj_w, in_proj_b, out_proj_w, out_proj_b):
    global LAST_RESULTS
    from concourse import bass_utils

    if "nc" not in _NC_CACHE:
        _NC_CACHE["nc"] = build_nc()
    nc = _NC_CACHE["nc"]

    in_maps = prep_inputs(x, ln_gamma, ln_beta, in_proj_w, in_proj_b,
                          out_proj_w, out_proj_b)
    res = bass_utils.run_bass_kernel_spmd(nc, in_maps, core_ids=list(range(N_CORES)))
    LAST_RESULTS = res
    out = np.stack([r["out"] for r in res.results], axis=0)
    return np.ascontiguousarray(out, dtype=np.float32)

